# revision 21
# baseline (speedup 1.0000x reference)
"""Bass/Tile kernel for causal multi-head attention block (nn_BlankAttention).

Sharding: 8 cores = 2 batches x 4 head-groups (4 heads each).
Each core computes, for its batch b and heads hg*4..hg*4+3:
  qkv projection, causal attention, partial output projection
  y_part = attn_out @ w_out_slice.  Host sums the 4 partials per batch.

v2 design (vs baseline):
  - Projection accumulates the K=2048 contraction in PSUM (16 chained
    matmuls per output tile) instead of SBUF round-trips; evacuations go
    to the Scalar engine (idle during proj).  Single xt stream feeds both
    the q/k tiles and the v tiles.
  - q/k/v and the exp'd score tiles are stored bf16 (halves SBUF, 2x DVE
    for mask muls; matmul rate for bf16 == fp32r so no PE cost).  The
    projection itself, the output projection and the softmax accumulators
    stay fp32/fp32r.
  - Attention interleaves the 4 heads' j-loops round-robin so 4 exps are
    always in flight and AV never waits on the Scalar engine.  All 4
    heads' softmax sums share one PSUM bank (rows 0/32/64/96).
  - The normalization chain (reciprocal -> cast -> broadcast-matmul ->
    OT mul) is deferred and injected into later blocks at points where
    its latency is hidden; l-tile order [2,3,0,1] keeps every deferred
    reciprocal clear of the next block's mask-muls on the in-order DVE
    queue.
  - Output projection runs as 16-group blocks between attention blocks.

Per-core DRAM tensors:
  xt    [2048, 2048]  x[b].T               (dmodel, tok)     fp32
  wqk   [2048, 1024]  w_in q/k cols        [q_h0|k_h0|q_h1|k_h1|...]
  wv    [2048,  512]  w_in v cols          [v_h0|v_h1|v_h2|v_h3]
  wout  [ 512, 2048]  w_out rows for the 4 heads (head-major)
  maskt [n_u,  128, 512]  mask tiles, 1.0 = allowed, 0.0 = masked (bf16)
  ones  [ 128,  128]  all ones (fp32)
  y     [2048, 2048]  output partial (tok, dmodel)   float32

schedule: list over l-tile i (4 tiles of 512 queries) of list of
  (j, mask_idx, lo) -- key tiles (128 keys); mask_idx -1 = no mask;
  lo = leading fully-masked query columns to skip (multiple of 128).
"""

import numpy as np
import concourse.bass as bass
import concourse.tile as tile
from concourse import bacc, mybir

S = 2048
DM = 2048
NHL = 4          # heads per core
DH = 128
SCALE = 1.0 / (DH ** 0.5)

F32 = mybir.dt.float32
F32R = mybir.dt.float32r
BF16 = mybir.dt.bfloat16
EXP = mybir.ActivationFunctionType.Exp


def build_nc(schedule, n_masks):
    nc = bacc.Bacc("TRN2", target_bir_lowering=False, debug=False, num_devices=8)
    xt_d = nc.dram_tensor("xt", [DM, S], BF16, kind="ExternalInput").ap()
    wqk_d = nc.dram_tensor("wqk", [DM, 2 * NHL * DH], BF16, kind="ExternalInput").ap()
    wv_d = nc.dram_tensor("wv", [DM, NHL * DH], BF16, kind="ExternalInput").ap()
    wout_d = nc.dram_tensor("wout", [NHL * DH, DM], F32R, kind="ExternalInput").ap()
    maskt_d = nc.dram_tensor("maskt", [n_masks, 128, 512], BF16, kind="ExternalInput").ap()
    ones_d = nc.dram_tensor("ones", [128, 128], F32R, kind="ExternalInput").ap()
    y_d = nc.dram_tensor("y", [S, DM], F32, kind="ExternalOutput").ap()

    with tile.TileContext(nc) as tc:
        with tc.tile_pool(name="pp", bufs=1) as pp:
            qkT = pp.tile([128, 8, S], BF16)       # [dh, 2h(q)|2h+1(k), tok]
            V = pp.tile([128, 16, 512], BF16)      # [tok%128, tok//128, vfeat]
            masks = pp.tile([128, n_masks, 512], BF16)
            ones_r = pp.tile([128, 128], F32R)
            ones_bf = pp.tile([128, 128], BF16)

            # ---- projection: single xt stream, PSUM k-accumulation ----
            evac_flip = [0]

            def evac(dst, src):
                # alternate engines so neither becomes the copy bottleneck
                if evac_flip[0] % 2 == 0:
                    nc.scalar.copy(dst, src)
                else:
                    nc.vector.tensor_copy(dst, src)
                evac_flip[0] += 1

            with tc.tile_pool(name="proj", bufs=1) as projp, \
                 tc.tile_pool(name="pps", bufs=1, space="PSUM") as pps:
                wvT = projp.tile([128, 16, 512], BF16)
                wqkT = projp.tile([128, 16, 1024], BF16)

                def qk_chain(xsl, ft, w):
                    ps = pps.tile([128, 512], F32, tag="pq", bufs=4,
                                  name=f"pq_w{w}f{ft}")
                    for dq in range(16):
                        nc.tensor.matmul(
                            ps[:], wqkT[:, dq, 128 * ft:128 * (ft + 1)], xsl(dq),
                            start=(dq == 0), stop=(dq == 15))
                    evac(qkT[:, ft, 512 * w:512 * (w + 1)], ps[:])

                def v_chain(xsl, sub, w):
                    ps2 = pps.tile([128, 512], F32, tag="pv", bufs=4,
                                   name=f"pv_w{w}s{sub}")
                    for dq in range(16):
                        nc.tensor.matmul(
                            ps2[:], xsl(dq, slice(128 * sub, 128 * (sub + 1))),
                            wvT[:, dq, :],
                            start=(dq == 0), stop=(dq == 15))
                    evac(V[:, 4 * w + sub, :], ps2[:])

                def make_xsl(halves):
                    def xsl(dq, cols=slice(None)):
                        return halves[dq // 8][:, dq % 8, cols]
                    return xsl

                def xt_tiles(w, per_slice):
                    halves = []
                    for hf in range(2):
                        xh = projp.tile([128, 8, 512], BF16, tag="xt", bufs=5,
                                        name=f"xt_w{w}h{hf}")
                        src = xt_d[1024 * hf:1024 * (hf + 1), 512 * w:512 * (w + 1)]
                        # always per-slice: the combined rearrange DMA takes
                        # ~3us to issue on the Sync engine and stalls the
                        # window's first chains
                        for dql in range(8):
                            nc.sync.dma_start(
                                xh[:, dql, :], src[128 * dql:128 * (dql + 1), :])
                        halves.append(xh)
                    return halves

                # Window 0 is DMA-bound: emit DMAs as per-dq (xt, wv, wqk-half)
                # triplets in exactly consumption order, and run 8 chains
                # dq-major so compute tracks DMA arrival.  wqk's second half +
                # the ft4-7 chains (also dq-major) follow.
                h0 = []
                for hf in range(2):
                    h0.append(projp.tile([128, 8, 512], BF16, tag="xt", bufs=5,
                                         name=f"xt_w0h{hf}"))
                for dq in range(16):
                    nc.sync.dma_start(
                        h0[dq // 8][:, dq % 8, :],
                        xt_d[128 * dq:128 * (dq + 1), 0:512])
                    nc.sync.dma_start(wvT[:, dq, :], wv_d[128 * dq:128 * (dq + 1), :])
                    nc.sync.dma_start(wqkT[:, dq, 0:512],
                                      wqk_d[128 * dq:128 * (dq + 1), 0:512])
                for dq in range(16):
                    nc.sync.dma_start(wqkT[:, dq, 512:1024],
                                      wqk_d[128 * dq:128 * (dq + 1), 512:1024])
                xsl0 = make_xsl(h0)
                pv0 = [pps.tile([128, 512], F32, tag="pv", bufs=4, name=f"pv0_{s}")
                       for s in range(4)]
                pq0 = [pps.tile([128, 512], F32, tag="pq", bufs=4, name=f"pq0_{f}")
                       for f in range(4)]
                for dq in range(16):
                    for sub in range(4):
                        nc.tensor.matmul(
                            pv0[sub][:], xsl0(dq, slice(128 * sub, 128 * (sub + 1))),
                            wvT[:, dq, :], start=(dq == 0), stop=(dq == 15))
                    for ft in range(4):
                        nc.tensor.matmul(
                            pq0[ft][:], wqkT[:, dq, 128 * ft:128 * (ft + 1)],
                            xsl0(dq), start=(dq == 0), stop=(dq == 15))
                for sub in range(4):
                    evac(V[:, sub, :], pv0[sub][:])
                for ft in range(4):
                    evac(qkT[:, ft, 0:512], pq0[ft][:])
                for ft in range(4, 8):
                    qk_chain(xsl0, ft, 0)

                for w in range(1, 4):
                    xsl = make_xsl(xt_tiles(w, per_slice=False))
                    for sub in range(4):
                        v_chain(xsl, sub, w)
                    for ft in range(8):
                        qk_chain(xsl, ft, w)

            # ---- attention + output projection ----
            with tc.tile_pool(name="attn", bufs=1) as ap, \
                 tc.tile_pool(name="aps", bufs=1, space="PSUM") as aps:
                OT = ap.tile([128, 4, S], F32R)       # [dh, h, tok] normalized
                woutT = ap.tile([128, 4, S], F32R)    # [dh, h, od]
                nc.sync.dma_start(woutT[:], wout_d.rearrange("(h p) o -> p h o", p=128))
                nc.sync.dma_start(masks[:], maskt_d.rearrange("u p c -> p u c"))
                nc.sync.dma_start(ones_r[:], ones_d[:])
                nc.vector.tensor_copy(ones_bf[:], ones_r[:])

                norm_state = {}   # i -> (s4rr, o_sbs)

                def attn_block(i, pending, inject_at=None, inject=None):
                    """Emit attention for l-tile i (4 heads round-robin).

                    pending: closures (prev block's PSUM->SBUF copies) emitted
                    after round 0 so they don't delay this block's first exps.
                    inject: closure emitted before round `inject_at` (the
                    deferred bc/OT-mul of an earlier l-tile, PE+DVE filler).
                    Returns this block's pending closures.
                    """
                    js = schedule[i]
                    nj = len(js)
                    # group adjacent full-width tiles: their exp'd tiles are
                    # tree-summed on the DVE (bf16 partials, quads in the big
                    # blocks) so one ones-matmul covers a whole group — cuts
                    # the PE rows spent on softmax denominators 2-4x.  PSUM
                    # accumulates the f32 group sums, so bf16 rounding stays
                    # on shallow (<=2 level) trees.
                    role = {}
                    p = 0
                    quad_ok = nj >= 12
                    while p < nj:
                        run = 0
                        while p + run < nj and js[p + run][2] == 0:
                            run += 1
                        if run == 0:
                            role[p] = ('single', None)
                            p += 1
                            continue
                        q = p
                        while run >= 4 and quad_ok:
                            role[q + 1] = ('qmid', (q, q + 1))
                            role[q + 3] = ('qend', (q, q + 1, q + 2, q + 3))
                            q += 4
                            run -= 4
                        while run >= 2:
                            role[q + 1] = ('pend', (q, q + 1))
                            q += 2
                            run -= 2
                        if run:
                            role[q] = ('single', None)
                            q += 1
                        p = q
                    oaccs = [aps.tile([128, 512], F32, tag=f"oacc{h}", bufs=1,
                                      name=f"oacc{h}_{i}") for h in range(4)]
                    sums4 = aps.tile([128, 512], F32, tag="sums4", bufs=1,
                                     name=f"sums4_{i}")
                    sums_started = [False] * 4

                    def scores(h, idx):
                        j, mi, lo = js[idx]
                        sc = aps.tile([128, 512], F32, tag="sc", bufs=3,
                                      name=f"sc{h}_{i}_{j}")
                        nc.tensor.matmul(
                            sc[:, lo:], qkT[:, 2 * h + 1, 128 * j:128 * (j + 1)],
                            qkT[:, 2 * h, 512 * i + lo:512 * (i + 1)],
                            start=True, stop=True)
                        ex = ap.tile([128, 512], BF16, tag="ex", bufs=10,
                                     name=f"ex{h}_{i}_{j}")
                        nc.scalar.activation(ex[:, lo:], sc[:, lo:], EXP, scale=SCALE)
                        if mi >= 0:
                            nc.vector.tensor_mul(ex[:, lo:], ex[:, lo:],
                                                 masks[:, mi, lo:])
                        return ex

                    def sum_mm(h, moving, lo, idx):
                        nc.tensor.matmul(
                            sums4[32 * h:32 * h + 1, lo:], ones_bf[:, 0:1],
                            moving[:, lo:],
                            start=(not sums_started[h]), stop=(idx == nj - 1),
                            tile_position=(0, 32 * h))
                        sums_started[h] = True

                    ex_hist = [[None] * nj for _ in range(4)]
                    pend1 = [None] * 4   # per-head first-pair partial of a quad

                    def tree_add(h, idx, a, b):
                        exs = ap.tile([128, 512], BF16, tag="exs", bufs=12,
                                      name=f"exs{h}_{i}_{idx}")
                        nc.vector.tensor_add(exs[:], a[:], b[:])
                        return exs

                    def accum(h, idx):
                        j, mi, lo = js[idx]
                        ex = ex_hist[h][idx]
                        nc.tensor.matmul(
                            oaccs[h][:, lo:], V[:, j, 128 * h:128 * (h + 1)],
                            ex[:, lo:],
                            start=(idx == 0), stop=(idx == nj - 1))
                        kind, grp = role.get(idx, (None, None))
                        if kind == 'single':
                            sum_mm(h, ex, lo, idx)
                        elif kind == 'pend':
                            exs = tree_add(h, idx, ex_hist[h][grp[0]], ex)
                            sum_mm(h, exs, 0, idx)
                        elif kind == 'qmid':
                            pend1[h] = tree_add(h, idx, ex_hist[h][grp[0]], ex)
                        elif kind == 'qend':
                            e2 = tree_add(h, idx, ex_hist[h][grp[2]], ex)
                            eq = tree_add(h, idx + 100, pend1[h], e2)
                            sum_mm(h, eq, 0, idx)

                    for idx in range(nj):
                        if inject is not None and idx == inject_at:
                            inject()
                            inject = None
                        for h in range(4):
                            ex_hist[h][idx] = scores(h, idx)
                            if idx > 0:
                                accum(h, idx - 1)
                        if idx == 0 and pending:
                            for fn in pending:
                                fn()
                            pending = None
                    if inject is not None:
                        inject()
                    for h in range(4):
                        accum(h, nj - 1)

                    # epilogue: sums evac now (gates the DVE reciprocal);
                    # oacc evacuations deferred into the next block.
                    s4 = ap.tile([128, 512], F32, tag="s4", bufs=2, name=f"s4_{i}")
                    nc.scalar.copy(s4[:], sums4[:])
                    s4r = ap.tile([128, 512], F32, tag="s4r", bufs=2, name=f"s4r_{i}")
                    nc.vector.reciprocal(s4r[:], s4[:])
                    o_sbs = [ap.tile([128, 512], F32, tag="osb", bufs=8,
                                     name=f"osb{h}_{i}") for h in range(4)]
                    norm_state[i] = (s4r, o_sbs)
                    new_pending = [
                        (lambda h=h: nc.scalar.copy(o_sbs[h][:], oaccs[h][:]))
                        for h in range(4)]
                    return new_pending

                def norm_pe(i):
                    """Deferred: broadcast 1/s via K=1 matmul, normalize OT."""
                    def fn():
                        s4r, o_sbs = norm_state.pop(i)
                        for h in range(4):
                            # copy 1/s to a partition-0 f32r row: walrus
                            # requires fmap and weight at the same partition
                            rtmp = ap.tile([1, 512], F32R, tag="rtmp", bufs=4,
                                           name=f"rtmp{h}_{i}")
                            nc.vector.tensor_copy(rtmp[:], s4r[32 * h:32 * h + 1, :])
                            bc = aps.tile([128, 512], F32, tag="sc", bufs=3,
                                          name=f"bc{h}_{i}")
                            nc.tensor.matmul(bc[:], ones_r[0:1, :], rtmp[:],
                                             start=True, stop=True)
                            nc.vector.tensor_mul(
                                OT[:, h, 512 * i:512 * (i + 1)], o_sbs[h][:], bc[:])
                    return fn

                def y_block(i, pending=None, inject_at=None, inject=None,
                            tail=False, dve_share=False):
                    g = 0
                    for tt in range(4 * i, 4 * i + 4):
                        for o in range(4):
                            if inject is not None and g == inject_at:
                                inject()
                                inject = None
                            # rotate across spare attention banks so psum
                            # recycling never gates the matmul stream
                            # first two groups stay on "sc": the pending
                            # o_sb copies that read the oacc banks are only
                            # flushed after group 2, so oacc reuse must not
                            # be emitted before them
                            yptag, ypb = [("sc", 3), ("sc", 3), ("oacc0", 1),
                                          ("oacc1", 1)][g % 4]
                            yp = aps.tile([128, 512], F32, tag=yptag, bufs=ypb,
                                          name=f"yp{tt}_{o}")
                            for h in range(4):
                                nc.tensor.matmul(
                                    yp[:], OT[:, h, 128 * tt:128 * (tt + 1)],
                                    woutT[:, h, 512 * o:512 * (o + 1)],
                                    start=(h == 0), stop=(h == 3))
                            ys = pp.tile([128, 512], F32, tag="ys", bufs=4,
                                         name=f"ys{tt}_{o}")
                            if (tail or dve_share) and g % 2 == 1:
                                nc.vector.tensor_copy(ys[:], yp[:])
                            else:
                                nc.scalar.copy(ys[:], yp[:])
                            nc.sync.dma_start(
                                y_d[128 * tt:128 * (tt + 1), 512 * o:512 * (o + 1)],
                                ys[:])
                            g += 1
                            if g == 2 and pending:
                                for fn in pending:
                                    fn()
                                pending = None
                    if inject is not None:
                        inject()

                # sequence: A2 A3 Y2 A0 Y3 A1 Y0 Y1
                p2 = attn_block(2, pending=None)
                p3 = attn_block(3, pending=p2,
                                inject_at=max(2, len(schedule[3]) - 6), inject=norm_pe(2))
                y_block(2, pending=p3, inject_at=8, inject=norm_pe(3))
                p0 = attn_block(0, pending=None)
                y_block(3, pending=p0, inject_at=8, inject=norm_pe(0), dve_share=True)
                p1 = attn_block(1, pending=None)
                y_block(0, pending=p1, inject_at=8, inject=norm_pe(1), dve_share=True)
                y_block(1, tail=True)
    nc.compile()
    return nc


def derive_schedule(mask):
    """mask: [S, S] bool, mask[l, L] True = masked (key L not visible to l).

    Returns (schedule, mask_tiles):
      schedule[i] = list of (j, mask_idx, lo) for l-tile i; mask_idx -1 = all
      allowed; lo = leading fully-masked query columns (multiple of 128).
      mask_tiles: [n_u, 128, 512] float32, allowed=1.0
    """
    schedule = []
    uniq = {}
    tiles = []
    for i in range(4):
        row = []
        for j in range(16):
            blk = mask[512 * i:512 * (i + 1), 128 * j:128 * (j + 1)]
            if blk.all():
                continue  # fully masked -> skip tile
            if not blk.any():
                row.append((j, -1, 0))
                continue
            t = (~blk.T).astype(np.float32)  # [L 128, l 512], allowed=1
            # leading fully-masked columns can be skipped; bf16 matmuls run
            # at full rate for any free size, so only keep 128 alignment
            nz = np.flatnonzero(t.any(axis=0))
            lo = min((int(nz[0]) if len(nz) else 0) // 128 * 128, 384)
            key = t.tobytes()
            if key not in uniq:
                uniq[key] = len(tiles)
                tiles.append(t)
            row.append((j, uniq[key], lo))
        schedule.append(row)
    if not tiles:
        tiles.append(np.ones((128, 512), np.float32))
    return schedule, np.stack(tiles)


def make_core_inputs(x, w_in, w_out, mask_tiles, b, hg):
    """Inputs for core handling batch b, heads hg*4..hg*4+3."""
    import ml_dtypes
    heads = range(hg * 4, hg * 4 + 4)
    xt = np.ascontiguousarray(x[b].T)
    wqk = np.concatenate(
        [w_in[:, h * 384 + o:h * 384 + o + 128] for h in heads for o in (0, 128)],
        axis=1)
    wv = np.concatenate([w_in[:, h * 384 + 256:h * 384 + 384] for h in heads], axis=1)
    wout = np.concatenate([w_out[h * 128:(h + 1) * 128, :] for h in heads], axis=0)
    return {
        "xt": np.ascontiguousarray(xt).astype(ml_dtypes.bfloat16),
        "wqk": np.ascontiguousarray(wqk).astype(ml_dtypes.bfloat16),
        "wv": np.ascontiguousarray(wv).astype(ml_dtypes.bfloat16),
        "wout": np.ascontiguousarray(wout, np.float32),
        "maskt": np.ascontiguousarray(mask_tiles).astype(ml_dtypes.bfloat16),
        "ones": np.ones((128, 128), np.float32),
    }


_CACHE = {}


def _get_nc(schedule, n_masks):
    key = (tuple(tuple(r) for r in schedule), n_masks)
    if key not in _CACHE:
        _CACHE[key] = build_nc(schedule, n_masks)
    return _CACHE[key]


def kernel(x, w_in, w_out, mask):
    """Full-input entry point: shards across 8 NeuronCores (batch x head-group),
    runs the Bass kernel SPMD, and reduces the per-core partial outputs."""
    from concourse import bass_utils
    x = np.ascontiguousarray(np.asarray(x), dtype=np.float32)
    w_in = np.ascontiguousarray(np.asarray(w_in), dtype=np.float32)
    w_out = np.ascontiguousarray(np.asarray(w_out), dtype=np.float32)
    B = x.shape[0]
    m2 = np.asarray(mask).reshape(S, S)
    schedule, mask_tiles = derive_schedule(m2)
    nc = _get_nc(schedule, mask_tiles.shape[0])
    in_maps = [make_core_inputs(x, w_in, w_out, mask_tiles, c // 4, c % 4)
               for c in range(8)]
    res = bass_utils.run_bass_kernel_spmd(nc, in_maps, core_ids=list(range(8)))
    y = np.zeros((B, S, DM), np.float32)
    for c in range(8):
        y[c // 4] += res.results[c]["y"]
    return y


# revision 24
# speedup vs baseline: 1.0174x; 1.0174x over previous
"""Bass/Tile kernel for causal multi-head attention block (nn_BlankAttention).

Sharding: 8 cores = 2 batches x 4 head-groups (4 heads each).
Each core computes, for its batch b and heads hg*4..hg*4+3:
  qkv projection, causal attention, partial output projection
  y_part = attn_out @ w_out_slice.  Host sums the 4 partials per batch.

v2 design (vs baseline):
  - Projection accumulates the K=2048 contraction in PSUM (16 chained
    matmuls per output tile) instead of SBUF round-trips; evacuations go
    to the Scalar engine (idle during proj).  Single xt stream feeds both
    the q/k tiles and the v tiles.
  - q/k/v and the exp'd score tiles are stored bf16 (halves SBUF, 2x DVE
    for mask muls; matmul rate for bf16 == fp32r so no PE cost).  The
    projection itself, the output projection and the softmax accumulators
    stay fp32/fp32r.
  - Attention interleaves the 4 heads' j-loops round-robin so 4 exps are
    always in flight and AV never waits on the Scalar engine.  All 4
    heads' softmax sums share one PSUM bank (rows 0/32/64/96).
  - The normalization chain (reciprocal -> cast -> broadcast-matmul ->
    OT mul) is deferred and injected into later blocks at points where
    its latency is hidden; l-tile order [2,3,0,1] keeps every deferred
    reciprocal clear of the next block's mask-muls on the in-order DVE
    queue.
  - Output projection runs as 16-group blocks between attention blocks.

Per-core DRAM tensors:
  xt    [2048, 2048]  x[b].T               (dmodel, tok)     fp32
  wqk   [2048, 1024]  w_in q/k cols        [q_h0|k_h0|q_h1|k_h1|...]
  wv    [2048,  512]  w_in v cols          [v_h0|v_h1|v_h2|v_h3]
  wout  [ 512, 2048]  w_out rows for the 4 heads (head-major)
  maskt [n_u,  128, 512]  mask tiles, 1.0 = allowed, 0.0 = masked (bf16)
  ones  [ 128,  128]  all ones (fp32)
  y     [2048, 2048]  output partial (tok, dmodel)   float32

schedule: list over l-tile i (4 tiles of 512 queries) of list of
  (j, mask_idx, lo) -- key tiles (128 keys); mask_idx -1 = no mask;
  lo = leading fully-masked query columns to skip (multiple of 128).
"""

import numpy as np
import concourse.bass as bass
import concourse.tile as tile
from concourse import bacc, mybir

S = 2048
DM = 2048
NHL = 4          # heads per core
DH = 128
SCALE = 1.0 / (DH ** 0.5)

F32 = mybir.dt.float32
F32R = mybir.dt.float32r
BF16 = mybir.dt.bfloat16
EXP = mybir.ActivationFunctionType.Exp


def build_nc(schedule, n_masks):
    nc = bacc.Bacc("TRN2", target_bir_lowering=False, debug=False, num_devices=8)
    xt_d = nc.dram_tensor("xt", [DM, S], BF16, kind="ExternalInput").ap()
    wqk_d = nc.dram_tensor("wqk", [DM, 2 * NHL * DH], BF16, kind="ExternalInput").ap()
    wv_d = nc.dram_tensor("wv", [DM, NHL * DH], BF16, kind="ExternalInput").ap()
    wout_d = nc.dram_tensor("wout", [NHL * DH, DM], F32R, kind="ExternalInput").ap()
    maskt_d = nc.dram_tensor("maskt", [n_masks, 128, 512], BF16, kind="ExternalInput").ap()
    ones_d = nc.dram_tensor("ones", [128, 128], F32R, kind="ExternalInput").ap()
    y_d = nc.dram_tensor("y", [S, DM], F32, kind="ExternalOutput").ap()

    with tile.TileContext(nc) as tc:
        with tc.tile_pool(name="pp", bufs=1) as pp:
            qkT = pp.tile([128, 8, S], BF16)       # [dh, 2h(q)|2h+1(k), tok]
            V = pp.tile([128, 16, 512], BF16)      # [tok%128, tok//128, vfeat]
            masks = pp.tile([128, n_masks, 512], BF16)
            ones_r = pp.tile([128, 128], F32R)
            ones_bf = pp.tile([128, 128], BF16)

            # ---- projection: single xt stream, PSUM k-accumulation ----
            evac_flip = [0]

            def evac(dst, src):
                # alternate engines so neither becomes the copy bottleneck
                if evac_flip[0] % 2 == 0:
                    nc.scalar.copy(dst, src)
                else:
                    nc.vector.tensor_copy(dst, src)
                evac_flip[0] += 1

            with tc.tile_pool(name="proj", bufs=1) as projp, \
                 tc.tile_pool(name="pps", bufs=1, space="PSUM") as pps:
                wvT = projp.tile([128, 16, 512], BF16)
                wqkT = projp.tile([128, 16, 1024], BF16)

                def qk_chain(xsl, ft, w):
                    ps = pps.tile([128, 512], F32, tag="pq", bufs=4,
                                  name=f"pq_w{w}f{ft}")
                    for dq in range(16):
                        nc.tensor.matmul(
                            ps[:], wqkT[:, dq, 128 * ft:128 * (ft + 1)], xsl(dq),
                            start=(dq == 0), stop=(dq == 15))
                    if w == 3:
                        # keep Scalar clear at the proj->attention seam: the
                        # first attention exps must not queue behind these
                        nc.vector.tensor_copy(qkT[:, ft, 512 * w:512 * (w + 1)],
                                              ps[:])
                    else:
                        evac(qkT[:, ft, 512 * w:512 * (w + 1)], ps[:])

                def v_chain(xsl, sub, w):
                    ps2 = pps.tile([128, 512], F32, tag="pv", bufs=4,
                                   name=f"pv_w{w}s{sub}")
                    for dq in range(16):
                        nc.tensor.matmul(
                            ps2[:], xsl(dq, slice(128 * sub, 128 * (sub + 1))),
                            wvT[:, dq, :],
                            start=(dq == 0), stop=(dq == 15))
                    evac(V[:, 4 * w + sub, :], ps2[:])

                def make_xsl(halves):
                    def xsl(dq, cols=slice(None)):
                        return halves[dq // 8][:, dq % 8, cols]
                    return xsl

                def xt_tiles(w, per_slice):
                    halves = []
                    for hf in range(2):
                        xh = projp.tile([128, 8, 512], BF16, tag="xt", bufs=5,
                                        name=f"xt_w{w}h{hf}")
                        src = xt_d[1024 * hf:1024 * (hf + 1), 512 * w:512 * (w + 1)]
                        # always per-slice: the combined rearrange DMA takes
                        # ~3us to issue on the Sync engine and can stall the
                        # window's first chains
                        for dql in range(8):
                            nc.sync.dma_start(
                                xh[:, dql, :], src[128 * dql:128 * (dql + 1), :])
                        halves.append(xh)
                    return halves

                # Window 0 is DMA-bound at the start: emit per-dq (xt, wv)
                # pairs in exactly consumption order and run only the 4
                # V-chains dq-major (supply-matched, dense from ~10us).  The
                # wqk stream lands behind them, so all 8 QK chains then run
                # chain-major on resident data with no mid-round stalls.
                h0 = []
                for hf in range(2):
                    h0.append(projp.tile([128, 8, 512], BF16, tag="xt", bufs=5,
                                         name=f"xt_w0h{hf}"))
                for dq in range(16):
                    nc.sync.dma_start(
                        h0[dq // 8][:, dq % 8, :],
                        xt_d[128 * dq:128 * (dq + 1), 0:512])
                    nc.sync.dma_start(wvT[:, dq, :], wv_d[128 * dq:128 * (dq + 1), :])
                for dq in range(16):
                    nc.sync.dma_start(wqkT[:, dq, 0:512],
                                      wqk_d[128 * dq:128 * (dq + 1), 0:512])
                for dq in range(16):
                    nc.sync.dma_start(wqkT[:, dq, 512:1024],
                                      wqk_d[128 * dq:128 * (dq + 1), 512:1024])
                xsl0 = make_xsl(h0)
                pv0 = [pps.tile([128, 512], F32, tag="pv", bufs=4, name=f"pv0_{s}")
                       for s in range(4)]
                for dq in range(16):
                    for sub in range(4):
                        nc.tensor.matmul(
                            pv0[sub][:], xsl0(dq, slice(128 * sub, 128 * (sub + 1))),
                            wvT[:, dq, :], start=(dq == 0), stop=(dq == 15))
                for sub in range(4):
                    evac(V[:, sub, :], pv0[sub][:])
                for ft in range(8):
                    qk_chain(xsl0, ft, 0)

                for w in range(1, 4):
                    xsl = make_xsl(xt_tiles(w, per_slice=False))
                    for sub in range(4):
                        v_chain(xsl, sub, w)
                    for ft in range(8):
                        qk_chain(xsl, ft, w)

            # ---- attention + output projection ----
            with tc.tile_pool(name="attn", bufs=1) as ap, \
                 tc.tile_pool(name="aps", bufs=1, space="PSUM") as aps:
                OT = ap.tile([128, 4, S], F32R)       # [dh, h, tok] normalized
                woutT = ap.tile([128, 4, S], F32R)    # [dh, h, od]
                nc.sync.dma_start(woutT[:], wout_d.rearrange("(h p) o -> p h o", p=128))
                nc.sync.dma_start(masks[:], maskt_d.rearrange("u p c -> p u c"))
                nc.sync.dma_start(ones_r[:], ones_d[:])
                nc.vector.tensor_copy(ones_bf[:], ones_r[:])

                norm_state = {}   # i -> (s4rr, o_sbs)

                def attn_block(i, pending, inject_at=None, inject=None):
                    """Emit attention for l-tile i (4 heads round-robin).

                    pending: closures (prev block's PSUM->SBUF copies) emitted
                    after round 0 so they don't delay this block's first exps.
                    inject: closure emitted before round `inject_at` (the
                    deferred bc/OT-mul of an earlier l-tile, PE+DVE filler).
                    Returns this block's pending closures.
                    """
                    js = schedule[i]
                    nj = len(js)
                    # group adjacent full-width tiles: their exp'd tiles are
                    # tree-summed on the DVE (bf16 partials, quads in the big
                    # blocks) so one ones-matmul covers a whole group — cuts
                    # the PE rows spent on softmax denominators 2-4x.  PSUM
                    # accumulates the f32 group sums, so bf16 rounding stays
                    # on shallow (<=2 level) trees.
                    role = {}
                    p = 0
                    quad_ok = nj >= 12
                    while p < nj:
                        run = 0
                        while p + run < nj and js[p + run][2] == 0:
                            run += 1
                        if run == 0:
                            role[p] = ('single', None)
                            p += 1
                            continue
                        q = p
                        while run >= 4 and quad_ok:
                            role[q + 1] = ('qmid', (q, q + 1))
                            role[q + 3] = ('qend', (q, q + 1, q + 2, q + 3))
                            q += 4
                            run -= 4
                        while run >= 2:
                            role[q + 1] = ('pend', (q, q + 1))
                            q += 2
                            run -= 2
                        if run:
                            role[q] = ('single', None)
                            q += 1
                        p = q
                    oaccs = [aps.tile([128, 512], F32, tag=f"oacc{h}", bufs=1,
                                      name=f"oacc{h}_{i}") for h in range(4)]
                    sums4 = aps.tile([128, 512], F32, tag="sums4", bufs=1,
                                     name=f"sums4_{i}")
                    sums_started = [False] * 4

                    def scores(h, idx):
                        j, mi, lo = js[idx]
                        sc = aps.tile([128, 512], F32, tag="sc", bufs=3,
                                      name=f"sc{h}_{i}_{j}")
                        nc.tensor.matmul(
                            sc[:, lo:], qkT[:, 2 * h + 1, 128 * j:128 * (j + 1)],
                            qkT[:, 2 * h, 512 * i + lo:512 * (i + 1)],
                            start=True, stop=True)
                        ex = ap.tile([128, 512], BF16, tag="ex", bufs=10,
                                     name=f"ex{h}_{i}_{j}")
                        nc.scalar.activation(ex[:, lo:], sc[:, lo:], EXP, scale=SCALE)
                        if mi >= 0:
                            nc.vector.tensor_mul(ex[:, lo:], ex[:, lo:],
                                                 masks[:, mi, lo:])
                        return ex

                    def sum_mm(h, moving, lo, idx):
                        nc.tensor.matmul(
                            sums4[32 * h:32 * h + 1, lo:], ones_bf[:, 0:1],
                            moving[:, lo:],
                            start=(not sums_started[h]), stop=(idx == nj - 1),
                            tile_position=(0, 32 * h))
                        sums_started[h] = True

                    ex_hist = [[None] * nj for _ in range(4)]
                    pend1 = [None] * 4   # per-head first-pair partial of a quad

                    def tree_add(h, idx, a, b):
                        exs = ap.tile([128, 512], BF16, tag="exs", bufs=12,
                                      name=f"exs{h}_{i}_{idx}")
                        nc.vector.tensor_add(exs[:], a[:], b[:])
                        return exs

                    def accum(h, idx):
                        j, mi, lo = js[idx]
                        ex = ex_hist[h][idx]
                        nc.tensor.matmul(
                            oaccs[h][:, lo:], V[:, j, 128 * h:128 * (h + 1)],
                            ex[:, lo:],
                            start=(idx == 0), stop=(idx == nj - 1))
                        kind, grp = role.get(idx, (None, None))
                        if kind == 'single':
                            sum_mm(h, ex, lo, idx)
                        elif kind == 'pend':
                            exs = tree_add(h, idx, ex_hist[h][grp[0]], ex)
                            sum_mm(h, exs, 0, idx)
                        elif kind == 'qmid':
                            pend1[h] = tree_add(h, idx, ex_hist[h][grp[0]], ex)
                        elif kind == 'qend':
                            e2 = tree_add(h, idx, ex_hist[h][grp[2]], ex)
                            eq = tree_add(h, idx + 100, pend1[h], e2)
                            sum_mm(h, eq, 0, idx)

                    for idx in range(nj):
                        if inject is not None and idx == inject_at:
                            inject()
                            inject = None
                        for h in range(4):
                            ex_hist[h][idx] = scores(h, idx)
                            if idx > 0:
                                accum(h, idx - 1)
                        if idx == 0 and pending:
                            for fn in pending:
                                fn()
                            pending = None
                    if inject is not None:
                        inject()
                    for h in range(4):
                        accum(h, nj - 1)

                    # epilogue: sums evac now (gates the DVE reciprocal);
                    # oacc evacuations deferred into the next block.
                    s4 = ap.tile([128, 512], F32, tag="s4", bufs=2, name=f"s4_{i}")
                    nc.scalar.copy(s4[:], sums4[:])
                    s4r = ap.tile([128, 512], F32, tag="s4r", bufs=2, name=f"s4r_{i}")
                    nc.vector.reciprocal(s4r[:], s4[:])
                    o_sbs = [ap.tile([128, 512], F32, tag="osb", bufs=8,
                                     name=f"osb{h}_{i}") for h in range(4)]
                    norm_state[i] = (s4r, o_sbs)
                    new_pending = [
                        (lambda h=h: nc.scalar.copy(o_sbs[h][:], oaccs[h][:]))
                        for h in range(4)]
                    return new_pending

                def norm_pe(i):
                    """Deferred: broadcast 1/s via K=1 matmul, normalize OT."""
                    def fn():
                        s4r, o_sbs = norm_state.pop(i)
                        for h in range(4):
                            # copy 1/s to a partition-0 f32r row: walrus
                            # requires fmap and weight at the same partition
                            rtmp = ap.tile([1, 512], F32R, tag="rtmp", bufs=4,
                                           name=f"rtmp{h}_{i}")
                            nc.vector.tensor_copy(rtmp[:], s4r[32 * h:32 * h + 1, :])
                            bc = aps.tile([128, 512], F32, tag="sc", bufs=3,
                                          name=f"bc{h}_{i}")
                            nc.tensor.matmul(bc[:], ones_r[0:1, :], rtmp[:],
                                             start=True, stop=True)
                            nc.vector.tensor_mul(
                                OT[:, h, 512 * i:512 * (i + 1)], o_sbs[h][:], bc[:])
                    return fn

                def y_block(i, pending=None, inject_at=None, inject=None,
                            tail=False):
                    g = 0
                    for tt in range(4 * i, 4 * i + 4):
                        for o in range(4):
                            if inject is not None and g == inject_at:
                                inject()
                                inject = None
                            # rotate across spare attention banks so psum
                            # recycling never gates the matmul stream
                            # first two groups stay on "sc": the pending
                            # o_sb copies that read the oacc banks are only
                            # flushed after group 2, so oacc reuse must not
                            # be emitted before them
                            yptag, ypb = [("sc", 3), ("sc", 3), ("oacc0", 1),
                                          ("oacc1", 1)][g % 4]
                            yp = aps.tile([128, 512], F32, tag=yptag, bufs=ypb,
                                          name=f"yp{tt}_{o}")
                            for h in range(4):
                                nc.tensor.matmul(
                                    yp[:], OT[:, h, 128 * tt:128 * (tt + 1)],
                                    woutT[:, h, 512 * o:512 * (o + 1)],
                                    start=(h == 0), stop=(h == 3))
                            ys = pp.tile([128, 512], F32, tag="ys", bufs=4,
                                         name=f"ys{tt}_{o}")
                            if tail and g % 2 == 1:
                                nc.vector.tensor_copy(ys[:], yp[:])
                            else:
                                nc.scalar.copy(ys[:], yp[:])
                            nc.sync.dma_start(
                                y_d[128 * tt:128 * (tt + 1), 512 * o:512 * (o + 1)],
                                ys[:])
                            g += 1
                            if g == 2 and pending:
                                for fn in pending:
                                    fn()
                                pending = None
                    if inject is not None:
                        inject()

                # sequence: A2 A3 Y2 A0 Y3 A1 Y0 Y1
                p2 = attn_block(2, pending=None)
                p3 = attn_block(3, pending=p2,
                                inject_at=max(2, len(schedule[3]) - 6), inject=norm_pe(2))
                y_block(2, pending=p3, inject_at=8, inject=norm_pe(3))
                p0 = attn_block(0, pending=None)
                y_block(3, pending=p0, inject_at=8, inject=norm_pe(0))
                p1 = attn_block(1, pending=None)
                y_block(0, pending=p1, inject_at=8, inject=norm_pe(1))
                y_block(1, tail=True)
    nc.compile()
    return nc


def derive_schedule(mask):
    """mask: [S, S] bool, mask[l, L] True = masked (key L not visible to l).

    Returns (schedule, mask_tiles):
      schedule[i] = list of (j, mask_idx, lo) for l-tile i; mask_idx -1 = all
      allowed; lo = leading fully-masked query columns (multiple of 128).
      mask_tiles: [n_u, 128, 512] float32, allowed=1.0
    """
    schedule = []
    uniq = {}
    tiles = []
    for i in range(4):
        row = []
        for j in range(16):
            blk = mask[512 * i:512 * (i + 1), 128 * j:128 * (j + 1)]
            if blk.all():
                continue  # fully masked -> skip tile
            if not blk.any():
                row.append((j, -1, 0))
                continue
            t = (~blk.T).astype(np.float32)  # [L 128, l 512], allowed=1
            # leading fully-masked columns can be skipped; bf16 matmuls run
            # at full rate for any free size, so only keep 128 alignment
            nz = np.flatnonzero(t.any(axis=0))
            lo = min((int(nz[0]) if len(nz) else 0) // 128 * 128, 384)
            key = t.tobytes()
            if key not in uniq:
                uniq[key] = len(tiles)
                tiles.append(t)
            row.append((j, uniq[key], lo))
        schedule.append(row)
    if not tiles:
        tiles.append(np.ones((128, 512), np.float32))
    return schedule, np.stack(tiles)


def make_core_inputs(x, w_in, w_out, mask_tiles, b, hg):
    """Inputs for core handling batch b, heads hg*4..hg*4+3."""
    import ml_dtypes
    heads = range(hg * 4, hg * 4 + 4)
    xt = np.ascontiguousarray(x[b].T)
    wqk = np.concatenate(
        [w_in[:, h * 384 + o:h * 384 + o + 128] for h in heads for o in (0, 128)],
        axis=1)
    wv = np.concatenate([w_in[:, h * 384 + 256:h * 384 + 384] for h in heads], axis=1)
    wout = np.concatenate([w_out[h * 128:(h + 1) * 128, :] for h in heads], axis=0)
    return {
        "xt": np.ascontiguousarray(xt).astype(ml_dtypes.bfloat16),
        "wqk": np.ascontiguousarray(wqk).astype(ml_dtypes.bfloat16),
        "wv": np.ascontiguousarray(wv).astype(ml_dtypes.bfloat16),
        "wout": np.ascontiguousarray(wout, np.float32),
        "maskt": np.ascontiguousarray(mask_tiles).astype(ml_dtypes.bfloat16),
        "ones": np.ones((128, 128), np.float32),
    }


_CACHE = {}


def _get_nc(schedule, n_masks):
    key = (tuple(tuple(r) for r in schedule), n_masks)
    if key not in _CACHE:
        _CACHE[key] = build_nc(schedule, n_masks)
    return _CACHE[key]


def kernel(x, w_in, w_out, mask):
    """Full-input entry point: shards across 8 NeuronCores (batch x head-group),
    runs the Bass kernel SPMD, and reduces the per-core partial outputs."""
    from concourse import bass_utils
    x = np.ascontiguousarray(np.asarray(x), dtype=np.float32)
    w_in = np.ascontiguousarray(np.asarray(w_in), dtype=np.float32)
    w_out = np.ascontiguousarray(np.asarray(w_out), dtype=np.float32)
    B = x.shape[0]
    m2 = np.asarray(mask).reshape(S, S)
    schedule, mask_tiles = derive_schedule(m2)
    nc = _get_nc(schedule, mask_tiles.shape[0])
    in_maps = [make_core_inputs(x, w_in, w_out, mask_tiles, c // 4, c % 4)
               for c in range(8)]
    res = bass_utils.run_bass_kernel_spmd(nc, in_maps, core_ids=list(range(8)))
    y = np.zeros((B, S, DM), np.float32)
    for c in range(8):
        y[c // 4] += res.results[c]["y"]
    return y


# revision 25
# speedup vs baseline: 1.0368x; 1.0191x over previous
"""Bass/Tile kernel for causal multi-head attention block (nn_BlankAttention).

Sharding: 8 cores = 2 batches x 4 head-groups (4 heads each).
Each core computes, for its batch b and heads hg*4..hg*4+3:
  qkv projection, causal attention, partial output projection
  y_part = attn_out @ w_out_slice.  Host sums the 4 partials per batch.

v2 design (vs baseline):
  - Projection accumulates the K=2048 contraction in PSUM (16 chained
    matmuls per output tile) instead of SBUF round-trips; evacuations go
    to the Scalar engine (idle during proj).  Single xt stream feeds both
    the q/k tiles and the v tiles.
  - q/k/v and the exp'd score tiles are stored bf16 (halves SBUF, 2x DVE
    for mask muls; matmul rate for bf16 == fp32r so no PE cost).  The
    projection itself, the output projection and the softmax accumulators
    stay fp32/fp32r.
  - Attention interleaves the 4 heads' j-loops round-robin so 4 exps are
    always in flight and AV never waits on the Scalar engine.  All 4
    heads' softmax sums share one PSUM bank (rows 0/32/64/96).
  - The normalization chain (reciprocal -> cast -> broadcast-matmul ->
    OT mul) is deferred and injected into later blocks at points where
    its latency is hidden; l-tile order [2,3,0,1] keeps every deferred
    reciprocal clear of the next block's mask-muls on the in-order DVE
    queue.
  - Output projection runs as 16-group blocks between attention blocks.

Per-core DRAM tensors:
  xt    [2048, 2048]  x[b].T               (dmodel, tok)     fp32
  wqk   [2048, 1024]  w_in q/k cols        [q_h0|k_h0|q_h1|k_h1|...]
  wv    [2048,  512]  w_in v cols          [v_h0|v_h1|v_h2|v_h3]
  wout  [ 512, 2048]  w_out rows for the 4 heads (head-major)
  maskt [n_u,  128, 512]  mask tiles, 1.0 = allowed, 0.0 = masked (bf16)
  ones  [ 128,  128]  all ones (fp32)
  y     [2048, 2048]  output partial (tok, dmodel)   float32

schedule: list over l-tile i (4 tiles of 512 queries) of list of
  (j, mask_idx, lo) -- key tiles (128 keys); mask_idx -1 = no mask;
  lo = leading fully-masked query columns to skip (multiple of 128).
"""

import numpy as np
import concourse.bass as bass
import concourse.tile as tile
from concourse import bacc, mybir

S = 2048
DM = 2048
NHL = 4          # heads per core
DH = 128
SCALE = 1.0 / (DH ** 0.5)

F32 = mybir.dt.float32
F32R = mybir.dt.float32r
BF16 = mybir.dt.bfloat16
EXP = mybir.ActivationFunctionType.Exp


def build_nc(schedule, n_masks):
    nc = bacc.Bacc("TRN2", target_bir_lowering=False, debug=False, num_devices=8)
    xt_d = nc.dram_tensor("xt", [DM, S], BF16, kind="ExternalInput").ap()
    wqk_d = nc.dram_tensor("wqk", [DM, 2 * NHL * DH], BF16, kind="ExternalInput").ap()
    wv_d = nc.dram_tensor("wv", [DM, NHL * DH], BF16, kind="ExternalInput").ap()
    wout_d = nc.dram_tensor("wout", [NHL * DH, DM], F32R, kind="ExternalInput").ap()
    maskt_d = nc.dram_tensor("maskt", [n_masks, 128, 512], BF16, kind="ExternalInput").ap()
    ones_d = nc.dram_tensor("ones", [128, 128], F32R, kind="ExternalInput").ap()
    y_d = nc.dram_tensor("y", [S, DM], F32, kind="ExternalOutput").ap()

    with tile.TileContext(nc) as tc:
        with tc.tile_pool(name="pp", bufs=1) as pp:
            qkT = pp.tile([128, 8, S], BF16)       # [dh, 2h(q)|2h+1(k), tok]
            V = pp.tile([128, 16, 512], BF16)      # [tok%128, tok//128, vfeat]
            masks = pp.tile([128, n_masks, 512], BF16)
            ones_r = pp.tile([128, 128], F32R)
            ones_bf = pp.tile([128, 128], BF16)

            # ---- projection: single xt stream, PSUM k-accumulation ----
            evac_flip = [0]

            def evac(dst, src):
                # alternate engines so neither becomes the copy bottleneck
                if evac_flip[0] % 2 == 0:
                    nc.scalar.copy(dst, src)
                else:
                    nc.vector.tensor_copy(dst, src)
                evac_flip[0] += 1

            with tc.tile_pool(name="proj", bufs=1) as projp, \
                 tc.tile_pool(name="pps", bufs=1, space="PSUM") as pps:
                wvT = projp.tile([128, 16, 512], BF16)
                wqkT = projp.tile([128, 16, 1024], BF16)

                def qk_chain(xsl, ft, w):
                    ps = pps.tile([128, 512], F32, tag="pq", bufs=4,
                                  name=f"pq_w{w}f{ft}")
                    for dq in range(16):
                        nc.tensor.matmul(
                            ps[:], wqkT[:, dq, 128 * ft:128 * (ft + 1)], xsl(dq),
                            start=(dq == 0), stop=(dq == 15))
                    if w == 3:
                        # keep Scalar clear at the proj->attention seam: the
                        # first attention exps must not queue behind these
                        nc.vector.tensor_copy(qkT[:, ft, 512 * w:512 * (w + 1)],
                                              ps[:])
                    else:
                        evac(qkT[:, ft, 512 * w:512 * (w + 1)], ps[:])

                def v_chain(xsl, sub, w):
                    ps2 = pps.tile([128, 512], F32, tag="pv", bufs=4,
                                   name=f"pv_w{w}s{sub}")
                    for dq in range(16):
                        nc.tensor.matmul(
                            ps2[:], xsl(dq, slice(128 * sub, 128 * (sub + 1))),
                            wvT[:, dq, :],
                            start=(dq == 0), stop=(dq == 15))
                    evac(V[:, 4 * w + sub, :], ps2[:])

                def make_xsl(halves):
                    def xsl(dq, cols=slice(None)):
                        return halves[dq // 8][:, dq % 8, cols]
                    return xsl

                def xt_tiles(w, per_slice):
                    halves = []
                    for hf in range(2):
                        xh = projp.tile([128, 8, 512], BF16, tag="xt", bufs=5,
                                        name=f"xt_w{w}h{hf}")
                        src = xt_d[1024 * hf:1024 * (hf + 1), 512 * w:512 * (w + 1)]
                        # always per-slice: the combined rearrange DMA takes
                        # ~3us to issue on the Sync engine and can stall the
                        # window's first chains
                        for dql in range(8):
                            nc.sync.dma_start(
                                xh[:, dql, :], src[128 * dql:128 * (dql + 1), :])
                        halves.append(xh)
                    return halves

                # Window 0 is DMA-bound at the start: emit per-dq (xt, wv)
                # pairs in exactly consumption order and run only the 4
                # V-chains dq-major (supply-matched, dense from ~10us).  The
                # wqk stream lands behind them, so all 8 QK chains then run
                # chain-major on resident data with no mid-round stalls.
                h0 = []
                for hf in range(2):
                    h0.append(projp.tile([128, 8, 512], BF16, tag="xt", bufs=5,
                                         name=f"xt_w0h{hf}"))
                for dq in range(16):
                    nc.sync.dma_start(
                        h0[dq // 8][:, dq % 8, :],
                        xt_d[128 * dq:128 * (dq + 1), 0:512])
                    nc.sync.dma_start(wvT[:, dq, :], wv_d[128 * dq:128 * (dq + 1), :])
                for dq in range(16):
                    nc.sync.dma_start(wqkT[:, dq, 0:512],
                                      wqk_d[128 * dq:128 * (dq + 1), 0:512])
                for dq in range(16):
                    nc.sync.dma_start(wqkT[:, dq, 512:1024],
                                      wqk_d[128 * dq:128 * (dq + 1), 512:1024])
                xsl0 = make_xsl(h0)
                pv0 = [pps.tile([128, 512], F32, tag="pv", bufs=4, name=f"pv0_{s}")
                       for s in range(4)]
                for dq in range(16):
                    for sub in range(4):
                        nc.tensor.matmul(
                            pv0[sub][:], xsl0(dq, slice(128 * sub, 128 * (sub + 1))),
                            wvT[:, dq, :], start=(dq == 0), stop=(dq == 15))
                for sub in range(4):
                    evac(V[:, sub, :], pv0[sub][:])
                for ft in range(8):
                    qk_chain(xsl0, ft, 0)

                for w in range(1, 4):
                    xsl = make_xsl(xt_tiles(w, per_slice=False))
                    for sub in range(4):
                        v_chain(xsl, sub, w)
                    for ft in range(8):
                        qk_chain(xsl, ft, w)

            # ---- attention + output projection ----
            with tc.tile_pool(name="attn", bufs=1) as ap, \
                 tc.tile_pool(name="aps", bufs=1, space="PSUM") as aps:
                OT = ap.tile([128, 4, S], F32R)       # [dh, h, tok] normalized
                woutT = ap.tile([128, 4, S], F32R)    # [dh, h, od]
                nc.sync.dma_start(woutT[:], wout_d.rearrange("(h p) o -> p h o", p=128))
                nc.sync.dma_start(masks[:], maskt_d.rearrange("u p c -> p u c"))
                nc.sync.dma_start(ones_r[:], ones_d[:])
                nc.vector.tensor_copy(ones_bf[:], ones_r[:])

                norm_state = {}   # i -> (s4rr, o_sbs)

                def attn_block(i, pending, inject_at=None, inject=None):
                    """Emit attention for l-tile i (4 heads round-robin).

                    pending: closures (prev block's PSUM->SBUF copies) emitted
                    after round 0 so they don't delay this block's first exps.
                    inject: closure emitted before round `inject_at` (the
                    deferred bc/OT-mul of an earlier l-tile, PE+DVE filler).
                    Returns this block's pending closures.
                    """
                    js = schedule[i]
                    nj = len(js)
                    # group adjacent full-width tiles: their exp'd tiles are
                    # tree-summed on the DVE (bf16 partials, quads in the big
                    # blocks) so one ones-matmul covers a whole group — cuts
                    # the PE rows spent on softmax denominators 2-4x.  PSUM
                    # accumulates the f32 group sums, so bf16 rounding stays
                    # on shallow (<=2 level) trees.
                    role = {}
                    p = 0
                    quad_ok = nj >= 12
                    while p < nj:
                        run = 0
                        while p + run < nj and js[p + run][2] == 0:
                            run += 1
                        if run == 0:
                            role[p] = ('single', None)
                            p += 1
                            continue
                        q = p
                        while run >= 4 and quad_ok:
                            role[q + 1] = ('qmid', (q, q + 1))
                            role[q + 3] = ('qend', (q, q + 1, q + 2, q + 3))
                            q += 4
                            run -= 4
                        while run >= 2:
                            role[q + 1] = ('pend', (q, q + 1))
                            q += 2
                            run -= 2
                        if run:
                            role[q] = ('single', None)
                            q += 1
                        p = q
                    oaccs = [aps.tile([128, 512], F32, tag=f"oacc{h}", bufs=1,
                                      name=f"oacc{h}_{i}") for h in range(4)]
                    sums4 = aps.tile([128, 512], F32, tag="sums4", bufs=1,
                                     name=f"sums4_{i}")
                    sums_started = [False] * 4

                    def scores(h, idx):
                        j, mi, lo = js[idx]
                        sc = aps.tile([128, 512], F32, tag="sc", bufs=3,
                                      name=f"sc{h}_{i}_{j}")
                        nc.tensor.matmul(
                            sc[:, lo:], qkT[:, 2 * h + 1, 128 * j:128 * (j + 1)],
                            qkT[:, 2 * h, 512 * i + lo:512 * (i + 1)],
                            start=True, stop=True)
                        ex = ap.tile([128, 512], BF16, tag="ex", bufs=10,
                                     name=f"ex{h}_{i}_{j}")
                        nc.scalar.activation(ex[:, lo:], sc[:, lo:], EXP, scale=SCALE)
                        if mi >= 0:
                            nc.vector.tensor_mul(ex[:, lo:], ex[:, lo:],
                                                 masks[:, mi, lo:])
                        return ex

                    def sum_mm(h, moving, lo, idx):
                        nc.tensor.matmul(
                            sums4[32 * h:32 * h + 1, lo:], ones_bf[:, 0:1],
                            moving[:, lo:],
                            start=(not sums_started[h]), stop=(idx == nj - 1),
                            tile_position=(0, 32 * h))
                        sums_started[h] = True

                    ex_hist = [[None] * nj for _ in range(4)]
                    pend1 = [None] * 4   # per-head first-pair partial of a quad

                    def tree_add(h, idx, a, b):
                        exs = ap.tile([128, 512], BF16, tag="exs", bufs=12,
                                      name=f"exs{h}_{i}_{idx}")
                        nc.vector.tensor_add(exs[:], a[:], b[:])
                        return exs

                    def accum(h, idx):
                        j, mi, lo = js[idx]
                        ex = ex_hist[h][idx]
                        nc.tensor.matmul(
                            oaccs[h][:, lo:], V[:, j, 128 * h:128 * (h + 1)],
                            ex[:, lo:],
                            start=(idx == 0), stop=(idx == nj - 1))
                        kind, grp = role.get(idx, (None, None))
                        if kind == 'single':
                            sum_mm(h, ex, lo, idx)
                        elif kind == 'pend':
                            exs = tree_add(h, idx, ex_hist[h][grp[0]], ex)
                            sum_mm(h, exs, 0, idx)
                        elif kind == 'qmid':
                            pend1[h] = tree_add(h, idx, ex_hist[h][grp[0]], ex)
                        elif kind == 'qend':
                            e2 = tree_add(h, idx, ex_hist[h][grp[2]], ex)
                            eq = tree_add(h, idx + 100, pend1[h], e2)
                            sum_mm(h, eq, 0, idx)

                    for idx in range(nj):
                        if inject is not None and idx == inject_at:
                            inject()
                            inject = None
                        for h in range(4):
                            ex_hist[h][idx] = scores(h, idx)
                            if idx > 0:
                                accum(h, idx - 1)
                        if idx == 0 and pending:
                            for fn in pending:
                                fn()
                            pending = None
                    if inject is not None:
                        inject()
                    for h in range(4):
                        accum(h, nj - 1)

                    # epilogue: sums evac now (gates the DVE reciprocal);
                    # oacc evacuations deferred into the next block.
                    s4 = ap.tile([128, 512], F32, tag="s4", bufs=2, name=f"s4_{i}")
                    nc.scalar.copy(s4[:], sums4[:])
                    s4r = ap.tile([128, 512], F32, tag="s4r", bufs=2, name=f"s4r_{i}")
                    nc.vector.reciprocal(s4r[:], s4[:])
                    o_sbs = [ap.tile([128, 512], F32, tag="osb", bufs=8,
                                     name=f"osb{h}_{i}") for h in range(4)]
                    norm_state[i] = (s4r, o_sbs)
                    new_pending = [
                        (lambda h=h: nc.scalar.copy(o_sbs[h][:], oaccs[h][:]))
                        for h in range(4)]
                    return new_pending

                def norm_pe(i):
                    """Deferred: broadcast 1/s via K=1 matmul, normalize OT."""
                    def fn():
                        s4r, o_sbs = norm_state.pop(i)
                        for h in range(4):
                            # copy 1/s to a partition-0 f32r row: walrus
                            # requires fmap and weight at the same partition
                            rtmp = ap.tile([1, 512], F32R, tag="rtmp", bufs=4,
                                           name=f"rtmp{h}_{i}")
                            nc.vector.tensor_copy(rtmp[:], s4r[32 * h:32 * h + 1, :])
                            bc = aps.tile([128, 512], F32, tag="sc", bufs=3,
                                          name=f"bc{h}_{i}")
                            nc.tensor.matmul(bc[:], ones_r[0:1, :], rtmp[:],
                                             start=True, stop=True)
                            nc.vector.tensor_mul(
                                OT[:, h, 512 * i:512 * (i + 1)], o_sbs[h][:], bc[:])
                    return fn

                def y_block(i, pending=None, inject_at=None, inject=None,
                            tail=False, dve_from=None):
                    g = 0
                    for tt in range(4 * i, 4 * i + 4):
                        for o in range(4):
                            if inject is not None and g == inject_at:
                                inject()
                                inject = None
                            # rotate across spare attention banks so psum
                            # recycling never gates the matmul stream
                            # first two groups stay on "sc": the pending
                            # o_sb copies that read the oacc banks are only
                            # flushed after group 2, so oacc reuse must not
                            # be emitted before them
                            yptag, ypb = [("sc", 3), ("sc", 3), ("oacc0", 1),
                                          ("oacc1", 1)][g % 4]
                            yp = aps.tile([128, 512], F32, tag=yptag, bufs=ypb,
                                          name=f"yp{tt}_{o}")
                            for h in range(4):
                                nc.tensor.matmul(
                                    yp[:], OT[:, h, 128 * tt:128 * (tt + 1)],
                                    woutT[:, h, 512 * o:512 * (o + 1)],
                                    start=(h == 0), stop=(h == 3))
                            ys = pp.tile([128, 512], F32, tag="ys", bufs=6,
                                         name=f"ys{tt}_{o}")
                            # alternate the trailing copies onto the DVE so
                            # the next block's scalar work (exps / psum
                            # rotation) doesn't queue behind them
                            if (tail or (dve_from is not None and g >= dve_from)) \
                                    and g % 2 == 1:
                                nc.vector.tensor_copy(ys[:], yp[:])
                            else:
                                nc.scalar.copy(ys[:], yp[:])
                            nc.sync.dma_start(
                                y_d[128 * tt:128 * (tt + 1), 512 * o:512 * (o + 1)],
                                ys[:])
                            g += 1
                            if g == 2 and pending:
                                for fn in pending:
                                    fn()
                                pending = None
                    if inject is not None:
                        inject()

                # sequence: A2 A3 Y2 A0 Y3 A1 Y0 Y1
                p2 = attn_block(2, pending=None)
                p3 = attn_block(3, pending=p2,
                                inject_at=max(2, len(schedule[3]) - 6), inject=norm_pe(2))
                y_block(2, pending=p3, inject_at=8, inject=norm_pe(3))
                p0 = attn_block(0, pending=None)
                y_block(3, pending=p0, inject_at=8, inject=norm_pe(0), dve_from=10)
                p1 = attn_block(1, pending=None)
                y_block(0, pending=p1, inject_at=8, inject=norm_pe(1), dve_from=10)
                y_block(1, tail=True)
    nc.compile()
    return nc


def derive_schedule(mask):
    """mask: [S, S] bool, mask[l, L] True = masked (key L not visible to l).

    Returns (schedule, mask_tiles):
      schedule[i] = list of (j, mask_idx, lo) for l-tile i; mask_idx -1 = all
      allowed; lo = leading fully-masked query columns (multiple of 128).
      mask_tiles: [n_u, 128, 512] float32, allowed=1.0
    """
    schedule = []
    uniq = {}
    tiles = []
    for i in range(4):
        row = []
        for j in range(16):
            blk = mask[512 * i:512 * (i + 1), 128 * j:128 * (j + 1)]
            if blk.all():
                continue  # fully masked -> skip tile
            if not blk.any():
                row.append((j, -1, 0))
                continue
            t = (~blk.T).astype(np.float32)  # [L 128, l 512], allowed=1
            # leading fully-masked columns can be skipped; bf16 matmuls run
            # at full rate for any free size, so only keep 128 alignment
            nz = np.flatnonzero(t.any(axis=0))
            lo = min((int(nz[0]) if len(nz) else 0) // 128 * 128, 384)
            key = t.tobytes()
            if key not in uniq:
                uniq[key] = len(tiles)
                tiles.append(t)
            row.append((j, uniq[key], lo))
        schedule.append(row)
    if not tiles:
        tiles.append(np.ones((128, 512), np.float32))
    return schedule, np.stack(tiles)


def make_core_inputs(x, w_in, w_out, mask_tiles, b, hg):
    """Inputs for core handling batch b, heads hg*4..hg*4+3."""
    import ml_dtypes
    heads = range(hg * 4, hg * 4 + 4)
    xt = np.ascontiguousarray(x[b].T)
    wqk = np.concatenate(
        [w_in[:, h * 384 + o:h * 384 + o + 128] for h in heads for o in (0, 128)],
        axis=1)
    wv = np.concatenate([w_in[:, h * 384 + 256:h * 384 + 384] for h in heads], axis=1)
    wout = np.concatenate([w_out[h * 128:(h + 1) * 128, :] for h in heads], axis=0)
    return {
        "xt": np.ascontiguousarray(xt).astype(ml_dtypes.bfloat16),
        "wqk": np.ascontiguousarray(wqk).astype(ml_dtypes.bfloat16),
        "wv": np.ascontiguousarray(wv).astype(ml_dtypes.bfloat16),
        "wout": np.ascontiguousarray(wout, np.float32),
        "maskt": np.ascontiguousarray(mask_tiles).astype(ml_dtypes.bfloat16),
        "ones": np.ones((128, 128), np.float32),
    }


_CACHE = {}


def _get_nc(schedule, n_masks):
    key = (tuple(tuple(r) for r in schedule), n_masks)
    if key not in _CACHE:
        _CACHE[key] = build_nc(schedule, n_masks)
    return _CACHE[key]


def kernel(x, w_in, w_out, mask):
    """Full-input entry point: shards across 8 NeuronCores (batch x head-group),
    runs the Bass kernel SPMD, and reduces the per-core partial outputs."""
    from concourse import bass_utils
    x = np.ascontiguousarray(np.asarray(x), dtype=np.float32)
    w_in = np.ascontiguousarray(np.asarray(w_in), dtype=np.float32)
    w_out = np.ascontiguousarray(np.asarray(w_out), dtype=np.float32)
    B = x.shape[0]
    m2 = np.asarray(mask).reshape(S, S)
    schedule, mask_tiles = derive_schedule(m2)
    nc = _get_nc(schedule, mask_tiles.shape[0])
    in_maps = [make_core_inputs(x, w_in, w_out, mask_tiles, c // 4, c % 4)
               for c in range(8)]
    res = bass_utils.run_bass_kernel_spmd(nc, in_maps, core_ids=list(range(8)))
    y = np.zeros((B, S, DM), np.float32)
    for c in range(8):
        y[c // 4] += res.results[c]["y"]
    return y
